# revision 7
# baseline (speedup 1.0000x reference)
"""TRN2 Bass kernel for nn_AugmentPipe: 3D affine warp + color + noise + cutout.

Self-contained: host-side parameter prep (jax CPU, replicating the
reference's RNG), a Tile/Bass kernel run SPMD on 8 NeuronCores (2 batches
per core), and gather/unshard logic.

Factorization used (exact): the reference's geometric transform only
rotates about z, so the trilinear warp = (per-slice 2D bilinear warp in
xy) composed with (1D linear resample along z). The z-resample and the
3x3 color matrix fuse into one Kronecker matmul on the TensorEngine:

  out[(i,z), j] = sum_{c,z'} Cm[i,c]*Mz[z,z'] * warped2d[(c,z'), j] + bias_i

The 2D warp runs as 4 corner gathers (GPSIMD indirect_copy, shared index
stream per batch) weighted on the VectorEngine. Noise (sigma-scaled,
host-generated to match jax threefry) is added by an SWDGE accumulate-DMA;
the cutout mask is a host-shipped 0/1 tile multiplied in on the
VectorEngine.
"""
import os
import sys
import types

import numpy as np

# ---------------------------------------------------------------- constants
XINT_MAX = 0.125; SCALE_STD = 0.2; ROTATE_MAX = 1.0; ANISO_STD = 0.2
XFRAC_STD = 0.125; BRIGHT_STD = 0.2; CONTRAST_STD = 0.5; HUE_MAX = 1.0
SAT_STD = 1.0; NOISE_STD = 0.1; CUTOUT_SIZE = 0.5

B, C, D, H, W = 16, 3, 64, 64, 64
NJ = H * W            # 4096 pixels per z-slice
NCORES = 8
BPC = B // NCORES     # batches per core = 2
NROW = BPC * C * D    # 384 in-rows per core
NBLK = 3              # 384 = 3 x 128
P = 128
HALF = NJ // 2        # 2048, combine granularity
ICC = 1024            # indirect_copy dst element limit
J5 = 512              # matmul free dim / PSUM bank

# ------------------------------------------------------------- axon shim


def _install_axon_shim():
    if "antenv.axon_hooks" not in sys.modules:
        mod = types.ModuleType("antenv.axon_hooks")
        mod._hook = None
        mod.set_axon_ntff_profile_hook = lambda h: setattr(mod, "_hook", h)
        mod.get_axon_ntff_profile_hook = lambda: mod._hook
        sys.modules["antenv.axon_hooks"] = mod
        try:
            from trn_agent_boot.trn_boot import _ntff_profile_via_ctypes
            mod._hook = _ntff_profile_via_ctypes("/opt/axon/libaxon_pjrt.so")
        except Exception:
            pass
    from concourse import bass_utils
    bass_utils.upload_artifacts = lambda tmpdir: tmpdir


def _legalize_sync_waits(nc, keep=1):
    """This container's walrus encodes at most ~1 sync-wait per instruction;
    hoist excess waits onto same-engine NoOps placed immediately before."""
    import bass_rust
    k = [0]
    for f in nc.m.functions:
        for bb in f.blocks:
            out = []
            for inst in bb.instructions:
                si = inst.sync_info
                waits = list(si.on_wait) if si is not None else []
                if len(waits) > keep:
                    head = waits[: len(waits) - keep]
                    tail = waits[len(waits) - keep:]
                    for wt in head:
                        nop = bass_rust.InstNoOp(
                            name=f"__waitnop_{k[0]}", ins=[], outs=[])
                        k[0] += 1
                        nop.engine = inst.engine
                        nop.sync_info = bass_rust.SyncInfo(
                            on_wait=[wt], on_update=[])
                        out.append(nop)
                    inst.sync_info = bass_rust.SyncInfo(
                        on_wait=tail, on_update=list(si.on_update))
                out.append(inst)
            bb.instructions = out
    return nc


# ------------------------------------------------------- host-side math

def _params():
    import jax
    import jax.numpy as jnp
    cpu = jax.devices("cpu")[0]
    with jax.default_device(cpu):
        ks = list(jax.random.split(jax.random.key(42), 24)); ki = iter(ks)
        rand = lambda s: np.asarray(jax.random.uniform(next(ki), s, dtype=jnp.float32))
        randn = lambda s: np.asarray(jax.random.normal(next(ki), s, dtype=jnp.float32))
        p = {}
        p["xflip"] = rand((B,)); p["rot90"] = rand((B,)); p["tint"] = rand((B, 3))
        p["iso"] = randn((B,)); p["prerot"] = rand((B,)); p["aniso"] = randn((B,))
        p["postrot"] = rand((B,)); p["tfrac"] = randn((B, 3))
        p["bright"] = randn((B,)); p["contrast"] = randn((B,))
        p["lumaflip"] = rand((B,)); p["hue"] = rand((B,)); p["sat"] = randn((B,))
        p["sigma_n"] = randn((B,)); p["noise"] = randn((B, C, D, H, W))
        p["center"] = rand((B, 3))
    return p


def _eye4(n):
    return np.tile(np.eye(4, dtype=np.float64), (n, 1, 1))


def _scale3d(sx, sy, sz):
    m = _eye4(len(sx)); m[:, 0, 0] = sx; m[:, 1, 1] = sy; m[:, 2, 2] = sz
    return m


def _translate3d(tx, ty, tz):
    m = _eye4(len(tx)); m[:, 0, 3] = tx; m[:, 1, 3] = ty; m[:, 2, 3] = tz
    return m


def _rotz(th):
    c, s = np.cos(th), np.sin(th)
    m = _eye4(len(th))
    m[:, 0, 0] = c; m[:, 0, 1] = -s; m[:, 1, 0] = s; m[:, 1, 1] = c
    return m


def _rot_luma(th):
    a = 1.0 / np.sqrt(3.0); a2 = 1.0 / 3.0
    c, s = np.cos(th), np.sin(th); cc = 1.0 - c
    d = a2 * cc + c; mm = a2 * cc - a * s; pp = a2 * cc + a * s
    m = _eye4(len(th))
    m[:, 0, 0] = d; m[:, 0, 1] = mm; m[:, 0, 2] = pp
    m[:, 1, 0] = pp; m[:, 1, 1] = d; m[:, 1, 2] = mm
    m[:, 2, 0] = mm; m[:, 2, 1] = pp; m[:, 2, 2] = d
    return m


def _geometry(p):
    ones = np.ones((B,))
    G = _eye4(B)
    G = G @ _scale3d(1 - 2 * np.floor(p["xflip"].astype(np.float64) * 2), ones, ones)
    G = G @ _rotz(-np.pi / 2 * np.floor(p["rot90"].astype(np.float64) * 4))
    t = (p["tint"].astype(np.float64) * 2 - 1) * XINT_MAX
    G = G @ _translate3d(-np.round(t[:, 0] * W), -np.round(t[:, 1] * H), -np.round(t[:, 2] * D))
    s = np.exp2(p["iso"].astype(np.float64) * SCALE_STD)
    G = G @ _scale3d(1 / s, 1 / s, 1 / s)
    G = G @ _rotz(-(p["prerot"].astype(np.float64) * 2 - 1) * np.pi * ROTATE_MAX)
    s = np.exp2(p["aniso"].astype(np.float64) * ANISO_STD)
    G = G @ _scale3d(1 / s, s, ones)
    G = G @ _rotz(-(p["postrot"].astype(np.float64) * 2 - 1) * np.pi * ROTATE_MAX)
    t = p["tfrac"].astype(np.float64) * XFRAC_STD
    G = G @ _translate3d(-t[:, 0] * W, -t[:, 1] * H, -t[:, 2] * D)
    return G


def _warp_tables(G):
    gx = np.arange(W, dtype=np.float64) - (W - 1) / 2.0
    gy = np.arange(H, dtype=np.float64) - (H - 1) / 2.0
    Y, X = np.meshgrid(gy, gx, indexing="ij")
    X = X.ravel(); Y = Y.ravel()
    idx = np.zeros((B, 4, NJ), dtype=np.int32)
    w = np.zeros((B, 4, NJ), dtype=np.float32)
    for b in range(B):
        sx = G[b, 0, 0] * X + G[b, 0, 1] * Y + G[b, 0, 3] + (W - 1) / 2.0
        sy = G[b, 1, 0] * X + G[b, 1, 1] * Y + G[b, 1, 3] + (H - 1) / 2.0
        x0 = np.floor(sx); y0 = np.floor(sy)
        wx = sx - x0; wy = sy - y0
        x0 = x0.astype(np.int64); y0 = y0.astype(np.int64)
        for k, (dy, dx) in enumerate(((0, 0), (0, 1), (1, 0), (1, 1))):
            xi = x0 + dx; yi = y0 + dy
            valid = (xi >= 0) & (xi < W) & (yi >= 0) & (yi < H)
            wk = (wx if dx else 1 - wx) * (wy if dy else 1 - wy) * valid
            xc = np.clip(xi, 0, W - 1); yc = np.clip(yi, 0, H - 1)
            idx[b, k] = (yc * W + xc).astype(np.int32)
            w[b, k] = wk.astype(np.float32)
    return idx, w


def _zinterp(G):
    gz = np.arange(D, dtype=np.float64) - (D - 1) / 2.0
    Mz = np.zeros((B, D, D), dtype=np.float32)
    for b in range(B):
        sz = G[b, 2, 2] * gz + G[b, 2, 3] + (D - 1) / 2.0
        z0 = np.floor(sz); wz = sz - z0
        z0i = z0.astype(np.int64)
        for z in range(D):
            for dz in (0, 1):
                zi = z0i[z] + dz
                if 0 <= zi < D:
                    Mz[b, z, zi] += np.float32(wz[z] if dz else 1 - wz[z])
    return Mz


def _colors(p):
    Cms = np.zeros((B, 4, 4))
    v = np.array([1, 1, 1, 0], dtype=np.float64) / np.sqrt(3.0)
    vv = np.outer(v, v); I4 = np.eye(4)
    b_ = p["bright"].astype(np.float64) * BRIGHT_STD
    c_ = np.exp2(p["contrast"].astype(np.float64) * CONTRAST_STD)
    lf = np.floor(p["lumaflip"].astype(np.float64) * 2)
    th = (p["hue"].astype(np.float64) * 2 - 1) * np.pi * HUE_MAX
    sat = np.exp2(p["sat"].astype(np.float64) * SAT_STD)
    rl = _rot_luma(th)
    for b in range(B):
        M = I4.copy()
        M = _translate3d(b_[b:b+1], b_[b:b+1], b_[b:b+1])[0] @ M
        M = _scale3d(c_[b:b+1], c_[b:b+1], c_[b:b+1])[0] @ M
        M = (I4 - 2 * vv * lf[b]) @ M
        M = rl[b] @ M
        M = (vv + (I4 - vv) * sat[b]) @ M
        Cms[b] = M
    return Cms


def _keep_mask(p):
    center = p["center"]
    half = np.float32(CUTOUT_SIZE / 2)
    fx = (np.arange(W, dtype=np.float32) + np.float32(0.5)) / np.float32(W)
    fy = (np.arange(H, dtype=np.float32) + np.float32(0.5)) / np.float32(H)
    fz = (np.arange(D, dtype=np.float32) + np.float32(0.5)) / np.float32(D)
    mx = np.abs(fx[None, :] - center[:, 0:1]) >= half
    my = np.abs(fy[None, :] - center[:, 1:2]) >= half
    mz = np.abs(fz[None, :] - center[:, 2:3]) >= half
    m = (mx[:, None, None, :] | my[:, None, :, None] | mz[:, :, None, None])
    return m.astype(np.float32)  # [B,D,H,W]


def _wrap_idx(flat):
    """[NJ] flat indices -> [128, NJ//16] u16 wrapped layout (all 8 groups)."""
    iw = np.zeros((16, NJ // 16), dtype=np.uint16)
    iw[np.arange(NJ) % 16, np.arange(NJ) // 16] = flat.astype(np.uint16)
    return np.tile(iw, (8, 1))


# in-row block layout per core: rows = [b0c0,b0c1 | b0c2,b1c2 | b1c0,b1c1] x z
# out-row block layout per core: rows = [b0i0,b0i1 | b0i2,b1i0 | b1i1,b1i2] x z
_IN_ROWS = [(0, 0), (0, 1), (0, 2), (1, 2), (1, 0), (1, 1)]   # (bslot, c) per 64-row group
_OUT_ROWS = [(0, 0), (0, 1), (0, 2), (1, 0), (1, 1), (1, 2)]  # (bslot, i)


def _prep_core_inputs(images, prep, core):
    """Build the per-core input arrays for batches [2*core, 2*core+1]."""
    idx, w, Mz, Cms = prep["idx"], prep["w"], prep["Mz"], prep["Cms"]
    mask, noise = prep["mask"], prep["noise_scaled"]
    b0, b1 = BPC * core, BPC * core + 1
    bs = (b0, b1)

    img = np.empty((NBLK, P, NJ), dtype=np.float32)
    for g, (sl, c) in enumerate(_IN_ROWS):
        img[g // 2, (g % 2) * 64:(g % 2) * 64 + 64] = \
            images[bs[sl], c].reshape(D, NJ)

    idxw = np.empty((4, NBLK, P, NJ // 16), dtype=np.uint16)
    for k in range(4):
        wrapped = {sl: _wrap_idx(idx[bs[sl], k]) for sl in (0, 1)}
        for g, (sl, _c) in enumerate(_IN_ROWS):
            idxw[k, g // 2, (g % 2) * 64:(g % 2) * 64 + 64] = \
                wrapped[sl][(g % 2) * 64:(g % 2) * 64 + 64]

    # weight rows [4 corners, 2 slots, NJ]
    wrows = np.stack([np.stack([w[bs[0], k], w[bs[1], k]]) for k in range(4)])
    wrows = np.ascontiguousarray(wrows, dtype=np.float32)

    # Kron lhsT per batch slot: lhsT[k=(c,z'), m=(i,z)] = Cm[i,c]*Mz[z,z']
    lhsT = np.empty((BPC, C * D, C * D), dtype=np.float32)
    for sl in (0, 1):
        K = np.einsum("ic,zt->ctiz", Cms[bs[sl], :3, :3].astype(np.float32),
                      Mz[bs[sl]]).reshape(C * D, C * D)
        lhsT[sl] = K
    # k1 (c2 rows) packed into one tile: rows 0:64 = slot0, 64:128 = slot1,
    # so the matmul lhsT base partition matches the rhs (block1 halves).
    lk1 = np.empty((P, C * D), dtype=np.float32)
    lk1[0:64] = lhsT[0][P:C * D]
    lk1[64:P] = lhsT[1][P:C * D]

    # bias per out-block column: [NBLK, P, 1]
    bias = np.empty((NBLK, P), dtype=np.float32)
    for g, (sl, i) in enumerate(_OUT_ROWS):
        bias[g // 2, (g % 2) * 64:(g % 2) * 64 + 64] = \
            np.float32(Cms[bs[sl], i, 3])

    # mask + noise in out-block layout [NBLK, P, NJ]
    mtiles = np.empty((NBLK, P, NJ), dtype=np.float32)
    ntiles = np.empty((NBLK, P, NJ), dtype=np.float32)
    for g, (sl, i) in enumerate(_OUT_ROWS):
        mtiles[g // 2, (g % 2) * 64:(g % 2) * 64 + 64] = \
            mask[bs[sl]].reshape(D, NJ)
        ntiles[g // 2, (g % 2) * 64:(g % 2) * 64 + 64] = \
            noise[bs[sl], i].reshape(D, NJ)

    # device layouts: idx as [P, 12*256] (corner-block along free),
    # bias as [P, NBLK]
    idx_dev = np.empty((P, 4 * NBLK * (NJ // 16)), dtype=np.uint16)
    for k in range(4):
        for t in range(NBLK):
            c0 = (k * NBLK + t) * (NJ // 16)
            idx_dev[:, c0:c0 + NJ // 16] = idxw[k, t]
    bias_dev = np.ascontiguousarray(bias.transpose(1, 0).reshape(P, NBLK),
                                    dtype=np.float32) if False else \
        np.ascontiguousarray(np.stack([bias[t] for t in range(NBLK)], axis=1))
    return {
        "img": img,
        "idxw": idx_dev,
        "wrows": wrows,
        "lhsT0": lhsT[0][0:P], "lhsT1": lhsT[1][0:P], "lhsTk1": lk1,
        "bias": bias_dev,
        "mtiles": mtiles, "ntiles": ntiles,
    }


def host_prepare(images):
    p = _params()
    G = _geometry(p)
    idx, w = _warp_tables(G)
    Mz = _zinterp(G)
    Cms = _colors(p)
    mask = _keep_mask(p)
    sigma = np.abs(p["sigma_n"]) * np.float32(NOISE_STD)
    # noise is added on-device AFTER the mask multiply, so fold the mask in
    noise_scaled = (p["noise"] * sigma[:, None, None, None, None]
                    * mask[:, None]).astype(np.float32)
    return dict(idx=idx, w=w, Mz=Mz, Cms=Cms, mask=mask,
                noise_scaled=noise_scaled)


# ------------------------------------------------------------ bass kernel

_BUILD_CACHE = {}


def build_bass():
    if "nc" in _BUILD_CACHE:
        return _BUILD_CACHE["nc"]
    import concourse.bass as bass
    import concourse.mybir as mybir
    import concourse.tile as tile

    F32 = mybir.dt.float32
    U16 = mybir.dt.uint16
    AF = mybir.ActivationFunctionType
    MUL = mybir.AluOpType.mult
    ADD = mybir.AluOpType.add

    nc = bass.Bass()
    img_in = nc.declare_dram_parameter("img", [NBLK, P, NJ], F32, isOutput=False)
    idx_in = nc.declare_dram_parameter("idxw", [P, 4 * NBLK * (NJ // 16)], U16, isOutput=False)
    w_in = nc.declare_dram_parameter("wrows", [4, BPC, NJ], F32, isOutput=False)
    l0_in = nc.declare_dram_parameter("lhsT0", [P, C * D], F32, isOutput=False)
    l1_in = nc.declare_dram_parameter("lhsT1", [P, C * D], F32, isOutput=False)
    lk1_in = nc.declare_dram_parameter("lhsTk1", [P, C * D], F32, isOutput=False)
    bias_in = nc.declare_dram_parameter("bias", [P, NBLK], F32, isOutput=False)
    m_in = nc.declare_dram_parameter("mtiles", [NBLK, P, NJ], F32, isOutput=False)
    n_in = nc.declare_dram_parameter("ntiles", [NBLK, P, NJ], F32, isOutput=False)
    out_d = nc.declare_dram_parameter("out", [NBLK, P, NJ], F32, isOutput=True)

    with tile.TileContext(nc) as tc:
        with (
            tc.tile_pool(name="cst", bufs=1) as cst,
            tc.tile_pool(name="raw", bufs=3) as rawp,
            tc.tile_pool(name="wrp", bufs=3) as wrpp,
            tc.tile_pool(name="gw", bufs=2) as gwp,
            tc.tile_pool(name="och", bufs=3) as ochp,
            tc.tile_pool(name="ps", bufs=2, space="PSUM") as psp,
        ):
            # ---- constants ----
            idxt = cst.tile([P, 4 * NBLK * (NJ // 16)], U16)
            nc.sync.dma_start(out=idxt, in_=idx_in[:, :])
            lhsT = []
            for sl, l_in in ((0, l0_in), (1, l1_in)):
                t0 = cst.tile([P, C * D], F32, tag=f"lk0_{sl}", name=f"lk0_{sl}")
                nc.sync.dma_start(out=t0, in_=l_in[0:P, :])
                lhsT.append(t0)
            lk1t = cst.tile([P, C * D], F32)
            nc.sync.dma_start(out=lk1t, in_=lk1_in[:, :])
            biast = cst.tile([P, NBLK], F32)
            nc.sync.dma_start(out=biast, in_=bias_in[:, :])

            # ---- raw loads ----
            raw = []
            for t in range(NBLK):
                rt = rawp.tile([P, NJ], F32, tag="raw")
                nc.sync.dma_start(out=rt, in_=img_in[t, :, :])
                raw.append(rt)

            # ---- 2D warp: gathers + weighted combine ----
            warped = []
            for t in range(NBLK):
                wt_full = wrpp.tile([P, NJ], F32, tag="warped")
                warped.append(wt_full)
            for t in range(NBLK):
                for h in range(NJ // HALF):
                    jlo = h * HALF
                    for k in range(4):
                        # gather corner k (two 1024-index chunks)
                        g = gwp.tile([P, HALF], F32, tag="g")
                        for q in range(HALF // ICC):
                            s0 = (jlo + q * ICC) // 16
                            nc.gpsimd.indirect_copy(
                                g[:, q * ICC:(q + 1) * ICC],
                                raw[t][:, :],
                                idxt[:, (k * NBLK + t) * (NJ // 16) + s0:
                                     (k * NBLK + t) * (NJ // 16) + s0 + ICC // 16],
                                True)
                        # weight tile (partition-broadcast DMA from rows)
                        wtile = gwp.tile([P, HALF], F32, tag="w")
                        if t == 0 or t == 2:
                            sl = 0 if t == 0 else 1
                            nc.scalar.dma_start(
                                out=wtile,
                                in_=w_in[k, sl:sl + 1, jlo:jlo + HALF].broadcast_to([P, HALF]))
                        else:
                            nc.scalar.dma_start(
                                out=wtile[0:64, :],
                                in_=w_in[k, 0:1, jlo:jlo + HALF].broadcast_to([64, HALF]))
                            nc.scalar.dma_start(
                                out=wtile[64:P, :],
                                in_=w_in[k, 1:2, jlo:jlo + HALF].broadcast_to([64, HALF]))
                        dst = warped[t][:, jlo:jlo + HALF]
                        if k == 0:
                            nc.vector.tensor_tensor(dst, g[:, :], wtile[:, :], op=MUL)
                        else:
                            nc.vector.tensor_tensor(g[:, :], g[:, :], wtile[:, :], op=MUL)
                            nc.vector.tensor_tensor(dst, dst, g[:, :], op=ADD)

            # ---- Kron matmul + eviction + mask + noise + store ----
            # psum pairs per J: blk0 full, blk1 split halves, blk2 full
            och = {}
            for t in range(NBLK):
                for h in range(NJ // HALF):
                    och[(t, h)] = ochp.tile([P, HALF], F32, tag="och", name=f"och_{t}_{h}")

            for jc in range(NJ // J5):
                jlo = jc * J5
                h = jlo // HALF
                jo = jlo - h * HALF
                # pair 0: out rows b0 m[0:128]
                ps0 = psp.tile([P, J5], F32, tag="ps0")
                nc.tensor.matmul(ps0[:, :], lhsT[0][:, 0:P],
                                 raw_rhs(warped, 0, jlo), start=True, stop=False)
                nc.tensor.matmul(ps0[:, :], lk1t[0:64, 0:P],
                                 warped[1][0:64, jlo:jlo + J5], start=False, stop=True)
                nc.scalar.activation(och[(0, h)][:, jo:jo + J5], ps0[:, :],
                                     AF.Identity, bias=biast[:, 0:1])
                # pair 1 top: b0 i2 -> psum[0:64]
                ps1a = psp.tile([64, J5], F32, tag="ps1a")
                nc.tensor.matmul(ps1a[:, :], lhsT[0][:, P:C * D],
                                 raw_rhs(warped, 0, jlo), start=True, stop=False)
                nc.tensor.matmul(ps1a[:, :], lk1t[0:64, P:C * D],
                                 warped[1][0:64, jlo:jlo + J5], start=False, stop=True)
                nc.scalar.activation(och[(1, h)][0:64, jo:jo + J5], ps1a[:, :],
                                     AF.Identity, bias=biast[0:64, 1:2])
                # pair 1 bottom: b1 i0 -> psum[64:128]
                ps1b = psp.tile([64, J5], F32, tag="ps1b")
                nc.tensor.matmul(ps1b[:, :], lhsT[1][:, 0:64],
                                 raw_rhs(warped, 2, jlo), start=True, stop=False)
                nc.tensor.matmul(ps1b[:, :], lk1t[64:P, 0:64],
                                 warped[1][64:P, jlo:jlo + J5], start=False, stop=True)
                nc.scalar.activation(och[(1, h)][64:P, jo:jo + J5], ps1b[:, :],
                                     AF.Identity, bias=biast[64:P, 1:2])
                # pair 2: b1 m[64:192]
                ps2 = psp.tile([P, J5], F32, tag="ps2")
                nc.tensor.matmul(ps2[:, :], lhsT[1][:, 64:C * D],
                                 raw_rhs(warped, 2, jlo), start=True, stop=False)
                nc.tensor.matmul(ps2[:, :], lk1t[64:P, 64:C * D],
                                 warped[1][64:P, jlo:jlo + J5], start=False, stop=True)
                nc.scalar.activation(och[(2, h)][:, jo:jo + J5], ps2[:, :],
                                     AF.Identity, bias=biast[:, 2:3])

            for t in range(NBLK):
                for h in range(NJ // HALF):
                    jlo = h * HALF
                    mt = gwp.tile([P, HALF], F32, tag="m")
                    nc.sync.dma_start(out=mt, in_=m_in[t, :, jlo:jlo + HALF])
                    oc = och[(t, h)]
                    nc.vector.tensor_tensor(oc[:, :], oc[:, :], mt[:, :], op=MUL)
                    nc.gpsimd.dma_start(out=oc[:, :],
                                        in_=n_in[t, :, jlo:jlo + HALF],
                                        accum_op=ADD)
                    nc.sync.dma_start(out=out_d[t, :, jlo:jlo + HALF], in_=oc[:, :])

    _BUILD_CACHE["nc"] = nc
    return nc


def raw_rhs(warped, t, jlo):
    return warped[t][:, jlo:jlo + J5]


# --------------------------------------------------------------- kernel()

def kernel(images: np.ndarray) -> np.ndarray:
    _install_axon_shim()
    from concourse.bass_utils import run_bass_kernel_spmd

    images = np.ascontiguousarray(np.asarray(images, dtype=np.float32))
    prep = host_prepare(images)

    nc = build_bass()
    if not _BUILD_CACHE.get("legalized"):
        _legalize_sync_waits(nc)
        _BUILD_CACHE["legalized"] = True

    in_maps = []
    for core in range(NCORES):
        ci = _prep_core_inputs(images, prep, core)
        ci["wrows"] = ci.pop("wrows")
        in_maps.append(ci)

    res = run_bass_kernel_spmd(nc, in_maps, core_ids=list(range(NCORES)),
                               trace=os.environ.get("KERNEL_TRACE", "0") == "1")
    _BUILD_CACHE["last_result"] = res

    out = np.empty((B, C, D, H, W), dtype=np.float32)
    for core in range(NCORES):
        o = res.results[core]["out"]          # [NBLK, P, NJ]
        bs = (BPC * core, BPC * core + 1)
        for g, (sl, i) in enumerate(_OUT_ROWS):
            out[bs[sl], i] = o[g // 2, (g % 2) * 64:(g % 2) * 64 + 64].reshape(D, H, W)
    return out
